# revision 16
# baseline (speedup 1.0000x reference)
"""AdaptiveMoDBlock Trainium2 kernel (8 NeuronCores, SPMD, no collectives).

Host (numpy): complexity head -> capacity -> k, router logits -> per-sequence
top-k token selection + sigmoid gates, gather.  The selected tokens (B*k of
them) are sharded evenly across the 8 cores; each core runs the inner FFN
(2048 -> 8192 GELU -> 2048, bf16 matmuls / fp32 accumulation) on its token
shard with the full weights streamed from HBM.  Host scatter-adds the gated
FFN outputs back onto the residual stream.

Device data layouts (per core, n = tokens per core):
  xt   bf16 [128, 16*n]     xt[p, t*n+j]      = selected[j, t*128+p]
  w1t  bf16 [64, 128, 2048] w1t[c, p, t*128+j] = w1[t*128+p, c*128+j]
  w2t  bf16 [16, 128, 8192] w2t[d, p, c*128+j] = w2[c*128+p, d*128+j]
  b1t  f32  [128, 64]       b1t[p, c]          = b1[c*128+p]
  b2t  f32  [128, 16]       b2t[p, d]          = b2[d*128+p]
  wtok f32  [128, n]        sigmoid gate per token, broadcast over partitions
  out  f32  [16, 128, n]    out[d, p, j]       = gate[j] * ffn(sel)[j, d*128+p]
"""

import math
import sys

import numpy as np

sys.path.insert(0, "/opt/trn_rl_repo")

import ml_dtypes  # noqa: E402

BF16 = ml_dtypes.bfloat16

B, S, D = 4, 4096, 2048
DC = D // 4
DFF = 4 * D
MIN_CAP, MAX_CAP = 0.25, 1.0

NCORES = 8
TPAD = 256          # per-core token count padded to this
TMAX = 512          # token tile width (= PSUM bank: 512 fp32, one matmul chain)

_cache = {}


def _token_tiles(n_tok):
    tiles = []
    rem = n_tok
    while rem > 0:
        t = TMAX if rem >= TMAX else rem
        tiles.append(t)
        rem -= t
    # a small tail tile halves the compute per streamed weight slab and
    # starves the PE on weight DMA; rebalance the last two tiles
    if len(tiles) >= 2 and tiles[-1] < 384:
        tot = tiles[-1] + tiles[-2]
        half = (tot // 2 + 63) // 64 * 64
        tiles[-2:] = [half, tot - half]
    return tiles


def _build(n_tok):
    """Build + compile the per-core FFN graph for n_tok tokens per core."""
    from concourse import bacc, mybir, tile

    F32 = mybir.dt.float32
    BF = mybir.dt.bfloat16
    AF = mybir.ActivationFunctionType

    tiles = _token_tiles(n_tok)

    nc = bacc.Bacc("TRN2", target_bir_lowering=False, debug=False,
                   num_devices=NCORES)
    # xt is laid out per token tile: [sum over tiles of 16*T] columns
    xt = nc.declare_dram_parameter("xt", [len(tiles), 128, 16 * TMAX], BF,
                                   isOutput=False)
    w1t = nc.declare_dram_parameter("w1t", [64, 128, 2048], BF, isOutput=False)
    w2t = nc.declare_dram_parameter("w2t", [16, 128, 8192], BF, isOutput=False)
    b1t = nc.declare_dram_parameter("b1t", [128, 64], F32, isOutput=False)
    b2t = nc.declare_dram_parameter("b2t", [128, 16], F32, isOutput=False)
    wtok = nc.declare_dram_parameter("wtok", [128, n_tok], F32, isOutput=False)
    out = nc.declare_dram_parameter("out", [16, 128, n_tok], F32, isOutput=True)

    with tile.TileContext(nc) as tc:
        with (
            tc.tile_pool(name="const", bufs=1) as cpool,
            tc.tile_pool(name="w1p", bufs=4) as w1p,
            tc.tile_pool(name="w2p", bufs=2) as w2p,
            tc.tile_pool(name="hp", bufs=1) as hp,
            tc.tile_pool(name="op", bufs=4) as op,
            tc.tile_pool(name="ph", bufs=4, space="PSUM") as ph,
            tc.tile_pool(name="po", bufs=3, space="PSUM") as po,
        ):
            # DMA ring split: w1 slabs ride the sync HWDGE ring, w2 slabs and
            # out tiles the scalar HWDGE ring, later activations/consts the
            # gpsimd SWDGE queues — weight-slab cadence never queues behind
            # the big activation transfers.  The first tile's activations are
            # split across both HW rings (behind the first two w1 slabs) so
            # the PE can start ~15us after launch.
            w1_pre = {}
            t_sb = w1p.tile([128, 2048], BF, tag="w1s")
            nc.sync.dma_start(t_sb[:], w1t[0])
            w1_pre[0] = t_sb
            xts = []
            for ti, T in enumerate(tiles):
                x_sb = cpool.tile([128, 16 * T], BF, tag=f"xts{ti}")
                if ti == 0:
                    # ramp-critical first-tile activations: scalar HW ring
                    # (the sync ring owns the w1 slab cadence)
                    nc.scalar.dma_start(x_sb[:], xt[ti, :, :16 * T])
                    t_sb = w1p.tile([128, 2048], BF, tag="w1s")
                    nc.sync.dma_start(t_sb[:], w1t[1])
                    w1_pre[1] = t_sb
                else:
                    nc.gpsimd.dma_start(x_sb[:], xt[ti, :, :16 * T])
                xts.append(x_sb)
                if ti == 0:
                    b1_sb = cpool.tile([128, 64], F32, tag="b1")
                    nc.gpsimd.dma_start(b1_sb[:], b1t[:])
            wtok_sb = cpool.tile([128, n_tok], F32, tag="wtok")
            nc.gpsimd.dma_start(wtok_sb[:], wtok[:])
            b2_sb = cpool.tile([128, 16], F32, tag="b2")
            nc.gpsimd.dma_start(b2_sb[:], b2t[:])

            off = 0
            for ti, T in enumerate(tiles):
                x_sb = xts[ti]
                h_sb = hp.tile([128, 64 * TMAX], BF, tag="h")
                # phase A: hT[c*128.., tok] = gelu(w1.T x + b1), per 128-wide
                # dff chunk c, accumulated over the 16 D-chunks t
                for c in range(64):
                    if ti == 0 and c in w1_pre:
                        w1_sb = w1_pre.pop(c)
                    else:
                        w1_sb = w1p.tile([128, 2048], BF, tag="w1s")
                        nc.sync.dma_start(w1_sb[:], w1t[c])
                    ps = ph.tile([128, TMAX], F32, tag="ps")
                    for t in range(16):
                        nc.tensor.matmul(
                            ps[:, :T],
                            w1_sb[:, t * 128:(t + 1) * 128],
                            x_sb[:, t * T: t * T + T],
                            start=(t == 0), stop=(t == 15),
                        )
                    nc.scalar.activation(
                        h_sb[:, c * T: c * T + T], ps[:, :T],
                        AF.Gelu, bias=b1_sb[:, c:c + 1])
                # phase B: out[d*128.., tok] = gate * (w2.T h + b2), per
                # 128-wide d chunk, accumulated over the 64 dff chunks c
                for d in range(16):
                    w2_sb = w2p.tile([128, 8192], BF, tag="w2s")
                    nc.scalar.dma_start(w2_sb[:], w2t[d])
                    pso = po.tile([128, TMAX], F32, tag="pso")
                    for c in range(64):
                        nc.tensor.matmul(
                            pso[:, :T],
                            w2_sb[:, c * 128:(c + 1) * 128],
                            h_sb[:, c * T: c * T + T],
                            start=(c == 0), stop=(c == 63),
                        )
                    o2_sb = op.tile([128, TMAX], F32, tag="o2")
                    nc.vector.scalar_tensor_tensor(
                        o2_sb[:, :T], pso[:, :T], b2_sb[:, d:d + 1],
                        wtok_sb[:, off: off + T],
                        op0=mybir.AluOpType.add, op1=mybir.AluOpType.mult)
                    nc.scalar.dma_start(out[d, :, off: off + T], o2_sb[:, :T])
                off += T

    nc.compile()
    return nc


def _gelu_exact(x):
    x = np.asarray(x, np.float32)
    erf = np.vectorize(math.erf, otypes=[np.float32])
    return (x * np.float32(0.5) *
            (np.float32(1.0) + erf(x.astype(np.float64) / math.sqrt(2.0))))


def _sigmoid(x):
    x64 = np.asarray(x, np.float64)
    return (1.0 / (1.0 + np.exp(-x64))).astype(np.float32)


def _route(hidden, router_weight, router_bias, comp_w1, comp_b1, comp_w2,
           comp_b2):
    """Host replica of the reference routing: returns (k, indices, gates)."""
    pooled = hidden.mean(axis=1, dtype=np.float32)               # [B, D]
    ch = _gelu_exact(pooled @ comp_w1 + comp_b1)                 # [B, DC]
    complexity = _sigmoid(ch @ comp_w2 + comp_b2)                # [B, 1]
    capacity = float(np.mean(np.float32(MIN_CAP) +
                             complexity * np.float32(MAX_CAP - MIN_CAP)))
    k = int(capacity * S)
    if k == 0:
        return 0, None, None
    logits = (hidden.reshape(-1, D) @ router_weight).reshape(B, S)
    logits = logits + router_bias[0]                             # [B, S]
    if k >= S:
        idx = np.broadcast_to(np.arange(S, dtype=np.int64), (B, S)).copy()
    else:
        idx = np.argpartition(logits, S - k, axis=1)[:, S - k:]  # [B, k]
    gates = _sigmoid(np.take_along_axis(logits, idx, axis=1))    # [B, k]
    return k, idx, gates


def _run(inputs, trace=False):
    from concourse.bass_utils import run_bass_kernel_spmd

    hidden = np.ascontiguousarray(np.asarray(inputs["hidden"], np.float32))
    router_weight = np.asarray(inputs["router_weight"], np.float32)
    router_bias = np.asarray(inputs["router_bias"], np.float32)
    comp_w1 = np.asarray(inputs["comp_w1"], np.float32)
    comp_b1 = np.asarray(inputs["comp_b1"], np.float32)
    comp_w2 = np.asarray(inputs["comp_w2"], np.float32)
    comp_b2 = np.asarray(inputs["comp_b2"], np.float32)
    ffn_w1 = np.asarray(inputs["ffn_w1"], np.float32)
    ffn_b1 = np.asarray(inputs["ffn_b1"], np.float32)
    ffn_w2 = np.asarray(inputs["ffn_w2"], np.float32)
    ffn_b2 = np.asarray(inputs["ffn_b2"], np.float32)

    k, idx, gates = _route(hidden, router_weight, router_bias, comp_w1,
                           comp_b1, comp_w2, comp_b2)
    if k == 0:
        return hidden.copy(), None

    ntot = B * k
    n_tok = -(-ntot // NCORES)             # per-core tokens
    n_tok = -(-n_tok // TPAD) * TPAD       # pad to TPAD multiple
    npad = NCORES * n_tok

    selected = np.take_along_axis(hidden, idx[:, :, None], axis=1)  # [B,k,D]
    tokens = np.zeros((npad, D), np.float32)
    tokens[:ntot] = selected.reshape(ntot, D)
    gate_flat = np.zeros((npad,), np.float32)
    gate_flat[:ntot] = gates.reshape(ntot)

    # per-core device arrays
    tokens_bf = tokens.astype(BF16)
    w1t = np.ascontiguousarray(
        ffn_w1.astype(BF16).reshape(16, 128, 64, 128)
        .transpose(2, 1, 0, 3)).reshape(64, 128, 2048)
    w2t = np.ascontiguousarray(
        ffn_w2.astype(BF16).reshape(64, 128, 16, 128)
        .transpose(2, 1, 0, 3)).reshape(16, 128, 8192)
    b1t = np.ascontiguousarray(ffn_b1.reshape(64, 128).T)
    b2t = np.ascontiguousarray(ffn_b2.reshape(16, 128).T)

    tiles = _token_tiles(n_tok)
    in_maps = []
    for c in range(NCORES):
        tok_c = tokens_bf[c * n_tok:(c + 1) * n_tok]             # [n, D]
        xt = np.zeros((len(tiles), 128, 16 * TMAX), BF16)
        o = 0
        for ti, T in enumerate(tiles):
            blk = tok_c[o:o + T].reshape(T, 16, 128).transpose(2, 1, 0)
            xt[ti, :, :16 * T] = blk.reshape(128, 16 * T)
            o += T
        wt = np.ascontiguousarray(np.broadcast_to(
            gate_flat[c * n_tok:(c + 1) * n_tok][None, :], (128, n_tok)))
        in_maps.append({"xt": xt, "w1t": w1t, "w2t": w2t, "b1t": b1t,
                        "b2t": b2t, "wtok": wt})

    if n_tok not in _cache:
        _cache[n_tok] = _build(n_tok)
    nc = _cache[n_tok]

    res = run_bass_kernel_spmd(nc, in_maps, core_ids=list(range(NCORES)),
                               trace=trace)

    weighted = np.empty((npad, D), np.float32)
    for c in range(NCORES):
        o = res.results[c]["out"]                                # [16,128,n]
        weighted[c * n_tok:(c + 1) * n_tok] = o.reshape(D, n_tok).T
    weighted = weighted[:ntot].reshape(B, k, D)

    output = hidden.copy()
    b_idx = np.arange(B)[:, None]
    output[b_idx, idx] += weighted
    return output, res.exec_time_ns


def kernel(**inputs):
    output, _ = _run(inputs, trace=False)
    return output


# revision 17
# speedup vs baseline: 1.1921x; 1.1921x over previous
"""AdaptiveMoDBlock Trainium2 kernel (8 NeuronCores, SPMD, no collectives).

Host (numpy): complexity head -> capacity -> k, router logits -> per-sequence
top-k token selection + sigmoid gates, gather.  The selected tokens (B*k of
them) are sharded evenly across the 8 cores; each core runs the inner FFN
(2048 -> 8192 GELU -> 2048, bf16 matmuls / fp32 accumulation) on its token
shard with the full weights streamed from HBM.  Host scatter-adds the gated
FFN outputs back onto the residual stream.

Device data layouts (per core, n = tokens per core):
  xt   bf16 [128, 16*n]     xt[p, t*n+j]      = selected[j, t*128+p]
  w1t  bf16 [64, 128, 2048] w1t[c, p, t*128+j] = w1[t*128+p, c*128+j]
  w2t  bf16 [16, 128, 8192] w2t[d, p, c*128+j] = w2[c*128+p, d*128+j]
  b1t  f32  [128, 64]       b1t[p, c]          = b1[c*128+p]
  b2t  f32  [128, 16]       b2t[p, d]          = b2[d*128+p]
  wtok f32  [128, n]        sigmoid gate per token, broadcast over partitions
  out  f32  [16, 128, n]    out[d, p, j]       = gate[j] * ffn(sel)[j, d*128+p]
"""

import math
import sys

import numpy as np

sys.path.insert(0, "/opt/trn_rl_repo")

import ml_dtypes  # noqa: E402

BF16 = ml_dtypes.bfloat16

B, S, D = 4, 4096, 2048
DC = D // 4
DFF = 4 * D
MIN_CAP, MAX_CAP = 0.25, 1.0

NCORES = 8
TPAD = 256          # per-core token count padded to this
TMAX = 512          # token tile width (= PSUM bank: 512 fp32, one matmul chain)

_cache = {}


def _token_tiles(n_tok):
    tiles = []
    rem = n_tok
    while rem > 0:
        t = TMAX if rem >= TMAX else rem
        tiles.append(t)
        rem -= t
    # a small tail tile halves the compute per streamed weight slab and
    # starves the PE on weight DMA; rebalance the last two tiles
    if len(tiles) >= 2 and tiles[-1] < 384:
        tot = tiles[-1] + tiles[-2]
        half = (tot // 2 + 63) // 64 * 64
        tiles[-2:] = [half, tot - half]
    return tiles


def _build(n_tok):
    """Build + compile the per-core FFN graph for n_tok tokens per core."""
    from concourse import bacc, mybir, tile

    F32 = mybir.dt.float32
    BF = mybir.dt.bfloat16
    AF = mybir.ActivationFunctionType

    tiles = _token_tiles(n_tok)

    nc = bacc.Bacc("TRN2", target_bir_lowering=False, debug=False,
                   num_devices=NCORES)
    # xt is laid out per token tile: [sum over tiles of 16*T] columns
    xt = nc.declare_dram_parameter("xt", [len(tiles), 128, 16 * TMAX], BF,
                                   isOutput=False)
    w1t = nc.declare_dram_parameter("w1t", [64, 128, 2048], BF, isOutput=False)
    w2t = nc.declare_dram_parameter("w2t", [16, 128, 8192], BF, isOutput=False)
    b1t = nc.declare_dram_parameter("b1t", [128, 64], F32, isOutput=False)
    b2t = nc.declare_dram_parameter("b2t", [128, 16], F32, isOutput=False)
    wtok = nc.declare_dram_parameter("wtok", [128, n_tok], F32, isOutput=False)
    out = nc.declare_dram_parameter("out", [16, 128, n_tok], F32, isOutput=True)

    with tile.TileContext(nc) as tc:
        with (
            tc.tile_pool(name="const", bufs=1) as cpool,
            tc.tile_pool(name="w1p", bufs=4) as w1p,
            tc.tile_pool(name="w2p", bufs=2) as w2p,
            tc.tile_pool(name="hp", bufs=1) as hp,
            tc.tile_pool(name="op", bufs=4) as op,
            tc.tile_pool(name="ph", bufs=4, space="PSUM") as ph,
            tc.tile_pool(name="po", bufs=3, space="PSUM") as po,
        ):
            # DMA ring split: w1 slabs ride the sync HWDGE ring, w2 slabs and
            # out tiles the scalar HWDGE ring, later activations/consts the
            # gpsimd SWDGE queues — weight-slab cadence never queues behind
            # the big activation transfers.  The first tile's activations are
            # split across both HW rings (behind the first two w1 slabs) so
            # the PE can start ~15us after launch.
            w1_pre = {}
            t_sb = w1p.tile([128, 2048], BF, tag="w1s")
            nc.sync.dma_start(t_sb[:], w1t[0])
            w1_pre[0] = t_sb
            xts = []
            for ti, T in enumerate(tiles):
                x_sb = cpool.tile([128, 16 * T], BF, tag=f"xts{ti}")
                if ti == 0:
                    # ramp-critical first-tile activations: scalar HW ring
                    # (the sync ring owns the w1 slab cadence)
                    nc.scalar.dma_start(x_sb[:], xt[ti, :, :16 * T])
                    t_sb = w1p.tile([128, 2048], BF, tag="w1s")
                    nc.sync.dma_start(t_sb[:], w1t[1])
                    w1_pre[1] = t_sb
                else:
                    nc.gpsimd.dma_start(x_sb[:], xt[ti, :, :16 * T])
                xts.append(x_sb)
                if ti == 0:
                    b1_sb = cpool.tile([128, 64], F32, tag="b1")
                    nc.gpsimd.dma_start(b1_sb[:], b1t[:])
            wtok_sb = cpool.tile([128, n_tok], F32, tag="wtok")
            nc.gpsimd.dma_start(wtok_sb[:], wtok[:])
            b2_sb = cpool.tile([128, 16], F32, tag="b2")
            nc.gpsimd.dma_start(b2_sb[:], b2t[:])

            off = 0
            for ti, T in enumerate(tiles):
                x_sb = xts[ti]
                h_sb = hp.tile([128, 64 * TMAX], BF, tag="h")
                # phase A: hT[c*128.., tok] = gelu(w1.T x + b1), per 128-wide
                # dff chunk c, accumulated over the 16 D-chunks t
                for c in range(64):
                    if ti == 0 and c in w1_pre:
                        w1_sb = w1_pre.pop(c)
                    else:
                        w1_sb = w1p.tile([128, 2048], BF, tag="w1s")
                        nc.sync.dma_start(w1_sb[:], w1t[c])
                    ps = ph.tile([128, TMAX], F32, tag="ps")
                    for t in range(16):
                        nc.tensor.matmul(
                            ps[:, :T],
                            w1_sb[:, t * 128:(t + 1) * 128],
                            x_sb[:, t * T: t * T + T],
                            start=(t == 0), stop=(t == 15),
                        )
                    nc.scalar.activation(
                        h_sb[:, c * T: c * T + T], ps[:, :T],
                        AF.Gelu, bias=b1_sb[:, c:c + 1])
                # phase B: out[d*128.., tok] = gate * (w2.T h + b2), per
                # 128-wide d chunk, accumulated over the 64 dff chunks c
                for d in range(16):
                    w2_sb = w2p.tile([128, 8192], BF, tag="w2s")
                    nc.scalar.dma_start(w2_sb[:], w2t[d])
                    pso = po.tile([128, TMAX], F32, tag="pso")
                    for c in range(64):
                        nc.tensor.matmul(
                            pso[:, :T],
                            w2_sb[:, c * 128:(c + 1) * 128],
                            h_sb[:, c * T: c * T + T],
                            start=(c == 0), stop=(c == 63),
                        )
                    o2_sb = op.tile([128, TMAX], F32, tag="o2")
                    nc.vector.scalar_tensor_tensor(
                        o2_sb[:, :T], pso[:, :T], b2_sb[:, d:d + 1],
                        wtok_sb[:, off: off + T],
                        op0=mybir.AluOpType.add, op1=mybir.AluOpType.mult)
                    nc.scalar.dma_start(out[d, :, off: off + T], o2_sb[:, :T])
                off += T

    nc.compile()
    return nc


def _gelu_exact(x):
    x = np.asarray(x, np.float32)
    erf = np.vectorize(math.erf, otypes=[np.float32])
    return (x * np.float32(0.5) *
            (np.float32(1.0) + erf(x.astype(np.float64) / math.sqrt(2.0))))


def _sigmoid(x):
    x64 = np.asarray(x, np.float64)
    return (1.0 / (1.0 + np.exp(-x64))).astype(np.float32)


def _route(hidden, router_weight, router_bias, comp_w1, comp_b1, comp_w2,
           comp_b2):
    """Host replica of the reference routing: returns (k, indices, gates)."""
    pooled = hidden.mean(axis=1, dtype=np.float32)               # [B, D]
    ch = _gelu_exact(pooled @ comp_w1 + comp_b1)                 # [B, DC]
    complexity = _sigmoid(ch @ comp_w2 + comp_b2)                # [B, 1]
    capacity = float(np.mean(np.float32(MIN_CAP) +
                             complexity * np.float32(MAX_CAP - MIN_CAP)))
    k = int(capacity * S)
    if k == 0:
        return 0, None, None
    logits = (hidden.reshape(-1, D) @ router_weight).reshape(B, S)
    logits = logits + router_bias[0]                             # [B, S]
    if k >= S:
        idx = np.broadcast_to(np.arange(S, dtype=np.int64), (B, S)).copy()
    else:
        idx = np.argpartition(logits, S - k, axis=1)[:, S - k:]  # [B, k]
    gates = _sigmoid(np.take_along_axis(logits, idx, axis=1))    # [B, k]
    return k, idx, gates


def _run(inputs, trace=False):
    from concourse.bass_utils import run_bass_kernel_spmd

    hidden = np.ascontiguousarray(np.asarray(inputs["hidden"], np.float32))
    router_weight = np.asarray(inputs["router_weight"], np.float32)
    router_bias = np.asarray(inputs["router_bias"], np.float32)
    comp_w1 = np.asarray(inputs["comp_w1"], np.float32)
    comp_b1 = np.asarray(inputs["comp_b1"], np.float32)
    comp_w2 = np.asarray(inputs["comp_w2"], np.float32)
    comp_b2 = np.asarray(inputs["comp_b2"], np.float32)
    ffn_w1 = np.asarray(inputs["ffn_w1"], np.float32)
    ffn_b1 = np.asarray(inputs["ffn_b1"], np.float32)
    ffn_w2 = np.asarray(inputs["ffn_w2"], np.float32)
    ffn_b2 = np.asarray(inputs["ffn_b2"], np.float32)

    k, idx, gates = _route(hidden, router_weight, router_bias, comp_w1,
                           comp_b1, comp_w2, comp_b2)
    if k == 0:
        return hidden.copy(), None

    ntot = B * k
    n_tok = -(-ntot // NCORES)             # per-core tokens
    n_tok = -(-n_tok // TPAD) * TPAD       # pad to TPAD multiple
    npad = NCORES * n_tok

    selected = np.take_along_axis(hidden, idx[:, :, None], axis=1)  # [B,k,D]
    tokens = np.zeros((npad, D), np.float32)
    tokens[:ntot] = selected.reshape(ntot, D)
    gate_flat = np.zeros((npad,), np.float32)
    gate_flat[:ntot] = gates.reshape(ntot)

    # per-core device arrays
    tokens_bf = tokens.astype(BF16)
    w1t = np.ascontiguousarray(
        ffn_w1.astype(BF16).reshape(16, 128, 64, 128)
        .transpose(2, 1, 0, 3)).reshape(64, 128, 2048)
    w2t = np.ascontiguousarray(
        ffn_w2.astype(BF16).reshape(64, 128, 16, 128)
        .transpose(2, 1, 0, 3)).reshape(16, 128, 8192)
    b1t = np.ascontiguousarray(ffn_b1.reshape(64, 128).T)
    b2t = np.ascontiguousarray(ffn_b2.reshape(16, 128).T)

    tiles = _token_tiles(n_tok)
    in_maps = []
    for c in range(NCORES):
        tok_c = tokens_bf[c * n_tok:(c + 1) * n_tok]             # [n, D]
        xt = np.zeros((len(tiles), 128, 16 * TMAX), BF16)
        o = 0
        for ti, T in enumerate(tiles):
            blk = tok_c[o:o + T].reshape(T, 16, 128).transpose(2, 1, 0)
            xt[ti, :, :16 * T] = blk.reshape(128, 16 * T)
            o += T
        wt = np.ascontiguousarray(np.broadcast_to(
            gate_flat[c * n_tok:(c + 1) * n_tok][None, :], (128, n_tok)))
        in_maps.append({"xt": xt, "w1t": w1t, "w2t": w2t, "b1t": b1t,
                        "b2t": b2t, "wtok": wt})

    if n_tok not in _cache:
        _cache[n_tok] = _build(n_tok)
    nc = _cache[n_tok]

    # the NRT occasionally reports a transient EXEC_UNIT_UNRECOVERABLE on
    # launch; a short-delay retry clears it
    last_err = None
    for attempt in range(3):
        try:
            res = run_bass_kernel_spmd(nc, in_maps,
                                       core_ids=list(range(NCORES)),
                                       trace=trace)
            break
        except Exception as e:  # noqa: BLE001
            last_err = e
            import time
            time.sleep(3.0 * (attempt + 1))
    else:
        raise last_err

    weighted = np.empty((npad, D), np.float32)
    for c in range(NCORES):
        o = res.results[c]["out"]                                # [16,128,n]
        weighted[c * n_tok:(c + 1) * n_tok] = o.reshape(D, n_tok).T
    weighted = weighted[:ntot].reshape(B, k, D)

    output = hidden.copy()
    b_idx = np.arange(B)[:, None]
    output[b_idx, idx] += weighted
    return output, res.exec_time_ns


def kernel(**inputs):
    output, _ = _run(inputs, trace=False)
    return output


# revision 19
# speedup vs baseline: 1.1953x; 1.0026x over previous
"""AdaptiveMoDBlock Trainium2 kernel (8 NeuronCores, SPMD, no collectives).

Host (numpy): complexity head -> capacity -> k, router logits -> per-sequence
top-k token selection + sigmoid gates, gather.  The selected tokens (B*k of
them) are sharded evenly across the 8 cores; each core runs the inner FFN
(2048 -> 8192 GELU -> 2048, bf16 matmuls / fp32 accumulation) on its token
shard with the full weights streamed from HBM.  Host scatter-adds the gated
FFN outputs back onto the residual stream.

Device data layouts (per core, n = tokens per core, token tiles of width T):
  xt   bf16 [ntiles, 128, 16*512]  per tile: xt[ti, p, t*T+j] = sel[off+j, t*128+p]
  w1t  bf16 [64, 128, 2048]        w1t[c, p, t*128+j] = w1[t*128+p, c*128+j]
  w2t  bf16 [16, 128, 8192]        w2t[d, p, c*128+j] = w2[c*128+p, d*128+j]
  b1t  f32  [128, 64]              b1t[p, c]          = b1[c*128+p]
  b2t  f32  [128, 16]              b2t[p, d]          = b2[d*128+p]
  wtok f32  [128, n]               sigmoid gate per token, broadcast over partitions
  out  f32  [16, 128, n]           out[d, p, j] = gate[j] * ffn(sel)[j, d*128+p]
"""

import math
import sys

import numpy as np

sys.path.insert(0, "/opt/trn_rl_repo")

import ml_dtypes  # noqa: E402

BF16 = ml_dtypes.bfloat16

B, S, D = 4, 4096, 2048
DC = D // 4
DFF = 4 * D
MIN_CAP, MAX_CAP = 0.25, 1.0

NCORES = 8
TPAD = 256          # per-core token count padded to this
TMAX = 512          # token tile width (= PSUM bank: 512 fp32, one matmul chain)

_cache = {}


def _token_tiles(n_tok):
    tiles = []
    rem = n_tok
    while rem > 0:
        t = TMAX if rem >= TMAX else rem
        tiles.append(t)
        rem -= t
    # a small tail tile halves the compute per streamed weight slab and
    # starves the PE on weight DMA; rebalance the last two tiles
    if len(tiles) >= 2 and tiles[-1] < 384:
        tot = tiles[-1] + tiles[-2]
        half = (tot // 2 + 63) // 64 * 64
        tiles[-2:] = [half, tot - half]
    return tiles


def _build(n_tok):
    """Build + compile the per-core FFN graph for n_tok tokens per core."""
    from concourse import bacc, mybir, tile

    F32 = mybir.dt.float32
    BF = mybir.dt.bfloat16
    AF = mybir.ActivationFunctionType

    tiles = _token_tiles(n_tok)

    nc = bacc.Bacc("TRN2", target_bir_lowering=False, debug=False,
                   num_devices=NCORES)
    # xt is laid out per token tile: [sum over tiles of 16*T] columns
    xt = nc.declare_dram_parameter("xt", [len(tiles), 128, 16 * TMAX], BF,
                                   isOutput=False)
    w1t = nc.declare_dram_parameter("w1t", [64, 128, 2048], BF, isOutput=False)
    w2t = nc.declare_dram_parameter("w2t", [16, 128, 8192], BF, isOutput=False)
    b1t = nc.declare_dram_parameter("b1t", [128, 64], F32, isOutput=False)
    b2t = nc.declare_dram_parameter("b2t", [128, 16], F32, isOutput=False)
    wtok = nc.declare_dram_parameter("wtok", [128, n_tok], F32, isOutput=False)
    out = nc.declare_dram_parameter("out", [16, 128, n_tok], F32, isOutput=True)

    with tile.TileContext(nc) as tc:
        with (
            tc.tile_pool(name="const", bufs=1) as cpool,
            tc.tile_pool(name="w1p", bufs=4) as w1p,
            tc.tile_pool(name="w2p", bufs=2) as w2p,
            tc.tile_pool(name="hp", bufs=1) as hp,
            tc.tile_pool(name="op", bufs=4) as op,
            tc.tile_pool(name="ph", bufs=4, space="PSUM") as ph,
            tc.tile_pool(name="po", bufs=3, space="PSUM") as po,
        ):
            # DMA ring split: w1 slabs ride the sync HWDGE ring, w2 slabs and
            # out tiles the scalar HWDGE ring, later activations/consts the
            # gpsimd SWDGE queues — weight-slab cadence never queues behind
            # the big activation transfers.  The first tile's activations are
            # split across both HW rings (behind the first two w1 slabs) so
            # the PE can start ~15us after launch.
            w1_pre = {}
            t_sb = w1p.tile([128, 2048], BF, tag="w1s")
            nc.sync.dma_start(t_sb[:], w1t[0])
            w1_pre[0] = t_sb
            xts = []
            for ti, T in enumerate(tiles):
                x_sb = cpool.tile([128, 16 * T], BF, tag=f"xts{ti}")
                if ti == 0:
                    # ramp-critical first-tile activations: scalar HW ring
                    # (the sync ring owns the w1 slab cadence)
                    nc.scalar.dma_start(x_sb[:], xt[ti, :, :16 * T])
                    t_sb = w1p.tile([128, 2048], BF, tag="w1s")
                    nc.sync.dma_start(t_sb[:], w1t[1])
                    w1_pre[1] = t_sb
                else:
                    nc.gpsimd.dma_start(x_sb[:], xt[ti, :, :16 * T])
                xts.append(x_sb)
                if ti == 0:
                    b1_sb = cpool.tile([128, 64], F32, tag="b1")
                    nc.gpsimd.dma_start(b1_sb[:], b1t[:])
            wtok_sb = cpool.tile([128, n_tok], F32, tag="wtok")
            nc.gpsimd.dma_start(wtok_sb[:], wtok[:])
            b2_sb = cpool.tile([128, 16], F32, tag="b2")
            nc.gpsimd.dma_start(b2_sb[:], b2t[:])

            off = 0
            for ti, T in enumerate(tiles):
                x_sb = xts[ti]
                h_sb = hp.tile([128, 64 * TMAX], BF, tag="h")
                # phase A: hT[c*128.., tok] = gelu(w1.T x + b1), per 128-wide
                # dff chunk c, accumulated over the 16 D-chunks t
                for c in range(64):
                    if ti == 0 and c in w1_pre:
                        w1_sb = w1_pre.pop(c)
                    else:
                        w1_sb = w1p.tile([128, 2048], BF, tag="w1s")
                        nc.sync.dma_start(w1_sb[:], w1t[c])
                    ps = ph.tile([128, TMAX], F32, tag="ps")
                    for t in range(16):
                        nc.tensor.matmul(
                            ps[:, :T],
                            w1_sb[:, t * 128:(t + 1) * 128],
                            x_sb[:, t * T: t * T + T],
                            start=(t == 0), stop=(t == 15),
                        )
                    nc.scalar.activation(
                        h_sb[:, c * T: c * T + T], ps[:, :T],
                        AF.Gelu, bias=b1_sb[:, c:c + 1])
                # phase B: out[d*128.., tok] = gate * (w2.T h + b2), per
                # 128-wide d chunk, accumulated over the 64 dff chunks c
                for d in range(16):
                    w2_sb = w2p.tile([128, 8192], BF, tag="w2s")
                    nc.scalar.dma_start(w2_sb[:], w2t[d])
                    pso = po.tile([128, TMAX], F32, tag="pso")
                    for c in range(64):
                        nc.tensor.matmul(
                            pso[:, :T],
                            w2_sb[:, c * 128:(c + 1) * 128],
                            h_sb[:, c * T: c * T + T],
                            start=(c == 0), stop=(c == 63),
                        )
                    o2_sb = op.tile([128, TMAX], F32, tag="o2")
                    nc.vector.scalar_tensor_tensor(
                        o2_sb[:, :T], pso[:, :T], b2_sb[:, d:d + 1],
                        wtok_sb[:, off: off + T],
                        op0=mybir.AluOpType.add, op1=mybir.AluOpType.mult)
                    nc.scalar.dma_start(out[d, :, off: off + T], o2_sb[:, :T])
                off += T

    nc.compile()
    return nc


def _gelu_exact(x):
    x = np.asarray(x, np.float32)
    erf = np.vectorize(math.erf, otypes=[np.float32])
    return (x * np.float32(0.5) *
            (np.float32(1.0) + erf(x.astype(np.float64) / math.sqrt(2.0))))


def _sigmoid(x):
    x64 = np.asarray(x, np.float64)
    return (1.0 / (1.0 + np.exp(-x64))).astype(np.float32)


def _route(hidden, router_weight, router_bias, comp_w1, comp_b1, comp_w2,
           comp_b2):
    """Host replica of the reference routing: returns (k, indices, gates)."""
    pooled = hidden.mean(axis=1, dtype=np.float32)               # [B, D]
    ch = _gelu_exact(pooled @ comp_w1 + comp_b1)                 # [B, DC]
    complexity = _sigmoid(ch @ comp_w2 + comp_b2)                # [B, 1]
    capacity = float(np.mean(np.float32(MIN_CAP) +
                             complexity * np.float32(MAX_CAP - MIN_CAP)))
    k = int(capacity * S)
    if k == 0:
        return 0, None, None
    logits = (hidden.reshape(-1, D) @ router_weight).reshape(B, S)
    logits = logits + router_bias[0]                             # [B, S]
    if k >= S:
        idx = np.broadcast_to(np.arange(S, dtype=np.int64), (B, S)).copy()
    else:
        idx = np.argpartition(logits, S - k, axis=1)[:, S - k:]  # [B, k]
    gates = _sigmoid(np.take_along_axis(logits, idx, axis=1))    # [B, k]
    return k, idx, gates


def _ensure_axon_hooks():
    """bass_utils imports antenv.axon_hooks unconditionally when the
    BASS_TRACE env var is set; supply a no-op hook registry if the module is
    absent so tracing degrades gracefully instead of crashing."""
    try:
        import antenv.axon_hooks  # noqa: F401
    except ImportError:
        import types
        mod = types.ModuleType("antenv.axon_hooks")
        mod._hook = None
        mod.set_axon_ntff_profile_hook = lambda h: setattr(mod, "_hook", h)
        mod.get_axon_ntff_profile_hook = lambda: mod._hook
        try:
            import antenv
            sys.modules["antenv.axon_hooks"] = mod
            antenv.axon_hooks = mod
        except ImportError:
            pass


def _run(inputs, trace=False):
    _ensure_axon_hooks()
    from concourse.bass_utils import run_bass_kernel_spmd

    hidden = np.ascontiguousarray(np.asarray(inputs["hidden"], np.float32))
    router_weight = np.asarray(inputs["router_weight"], np.float32)
    router_bias = np.asarray(inputs["router_bias"], np.float32)
    comp_w1 = np.asarray(inputs["comp_w1"], np.float32)
    comp_b1 = np.asarray(inputs["comp_b1"], np.float32)
    comp_w2 = np.asarray(inputs["comp_w2"], np.float32)
    comp_b2 = np.asarray(inputs["comp_b2"], np.float32)
    ffn_w1 = np.asarray(inputs["ffn_w1"], np.float32)
    ffn_b1 = np.asarray(inputs["ffn_b1"], np.float32)
    ffn_w2 = np.asarray(inputs["ffn_w2"], np.float32)
    ffn_b2 = np.asarray(inputs["ffn_b2"], np.float32)

    k, idx, gates = _route(hidden, router_weight, router_bias, comp_w1,
                           comp_b1, comp_w2, comp_b2)
    if k == 0:
        return hidden.copy(), None

    ntot = B * k
    n_tok = -(-ntot // NCORES)             # per-core tokens
    n_tok = -(-n_tok // TPAD) * TPAD       # pad to TPAD multiple
    npad = NCORES * n_tok

    selected = np.take_along_axis(hidden, idx[:, :, None], axis=1)  # [B,k,D]
    tokens = np.zeros((npad, D), np.float32)
    tokens[:ntot] = selected.reshape(ntot, D)
    gate_flat = np.zeros((npad,), np.float32)
    gate_flat[:ntot] = gates.reshape(ntot)

    # per-core device arrays
    tokens_bf = tokens.astype(BF16)
    w1t = np.ascontiguousarray(
        ffn_w1.astype(BF16).reshape(16, 128, 64, 128)
        .transpose(2, 1, 0, 3)).reshape(64, 128, 2048)
    w2t = np.ascontiguousarray(
        ffn_w2.astype(BF16).reshape(64, 128, 16, 128)
        .transpose(2, 1, 0, 3)).reshape(16, 128, 8192)
    b1t = np.ascontiguousarray(ffn_b1.reshape(64, 128).T)
    b2t = np.ascontiguousarray(ffn_b2.reshape(16, 128).T)

    tiles = _token_tiles(n_tok)
    in_maps = []
    for c in range(NCORES):
        tok_c = tokens_bf[c * n_tok:(c + 1) * n_tok]             # [n, D]
        xt = np.zeros((len(tiles), 128, 16 * TMAX), BF16)
        o = 0
        for ti, T in enumerate(tiles):
            blk = tok_c[o:o + T].reshape(T, 16, 128).transpose(2, 1, 0)
            xt[ti, :, :16 * T] = blk.reshape(128, 16 * T)
            o += T
        wt = np.ascontiguousarray(np.broadcast_to(
            gate_flat[c * n_tok:(c + 1) * n_tok][None, :], (128, n_tok)))
        in_maps.append({"xt": xt, "w1t": w1t, "w2t": w2t, "b1t": b1t,
                        "b2t": b2t, "wtok": wt})

    if n_tok not in _cache:
        _cache[n_tok] = _build(n_tok)
    nc = _cache[n_tok]

    # the NRT occasionally reports a transient EXEC_UNIT_UNRECOVERABLE on
    # launch; a short-delay retry clears it
    last_err = None
    for attempt in range(3):
        try:
            res = run_bass_kernel_spmd(nc, in_maps,
                                       core_ids=list(range(NCORES)),
                                       trace=trace)
            break
        except Exception as e:  # noqa: BLE001
            last_err = e
            import time
            time.sleep(3.0 * (attempt + 1))
    else:
        raise last_err

    weighted = np.empty((npad, D), np.float32)
    for c in range(NCORES):
        o = res.results[c]["out"]                                # [16,128,n]
        weighted[c * n_tok:(c + 1) * n_tok] = o.reshape(D, n_tok).T
    weighted = weighted[:ntot].reshape(B, k, D)

    output = hidden.copy()
    b_idx = np.arange(B)[:, None]
    output[b_idx, idx] += weighted
    return output, res.exec_time_ns


def kernel(**inputs):
    output, _ = _run(inputs, trace=False)
    return output
